# revision 2
# baseline (speedup 1.0000x reference)
"""Trainium2 Bass kernel for nn_BitfieldLinear (vq_codebook).

Reference computation:
    idx   = codes & 0xFF            (basis row, 256 entries)
    r_q   = (codes >> 8) & 0xFFF
    sign  = bit20 ? -1 : +1
    scale = sign * tanh(r_q / 4095)
    W     = scale[:, None] * basis[idx]        # [8192, 4096]
    y     = x @ W.T                            # [128, 8192]

Key factorization (never materialize the 128MB W):
    Z = x @ basis.T                            # [128, 256]  tiny matmul
    y[b, j] = scale[j] * Z[b, idx[j]]          # column gather + scale

Sharding: out_features column-parallel across 8 cores (1024 codes per
core); x and basis replicated.  Per core:
    1. DMA x^T / basis^T K-tiled (host pre-laid-out for contiguous DMA)
    2. decode its 1024 codes on-chip (DVE bitops + ACT tanh)
    3. 64 fp32 matmuls accumulate Z^T [256, 128] in PSUM
    4. Z^T -> DRAM scratch, dma_gather 1024 rows of 512B by idx
    5. per-row scale multiply, store [128, 8, 128] out
Host reassembles y from the 8 per-core outputs (pure layout transform).
"""

import sys

for _p in ("/opt/trn_rl_repo", "/opt/pypackages"):
    if _p not in sys.path:
        sys.path.insert(0, _p)

import numpy as np

import concourse.bacc as bacc
import concourse.mybir as mybir
import concourse.tile as tile
from concourse.alu_op_type import AluOpType
from concourse.bass_utils import run_bass_kernel_spmd

N_CORES = 8
BATCH = 128
IN_F = 4096
OUT_F = 8192
BASIS = 256
OPC = OUT_F // N_CORES      # 1024 output columns per core
NK = IN_F // 128            # 32 K-tiles
NT = OPC // 128             # 8 gathered row-tiles per core
R_LEVELS = 4095.0

F32 = mybir.dt.float32
I32 = mybir.dt.int32
I16 = mybir.dt.int16

# K-tiles per DMA chunk for load/compute overlap
DMA_CHUNK = 8


def build_nc():
    nc = bacc.Bacc(
        "TRN2",
        target_bir_lowering=False,
        debug=False,
        num_devices=N_CORES,
    )

    xt_d = nc.dram_tensor("xt", [128, IN_F], F32, kind="ExternalInput")
    bt_d = nc.dram_tensor("bt", [128, 2 * IN_F], F32, kind="ExternalInput")
    c16_d = nc.dram_tensor("c16", [128, OPC // 16], I32, kind="ExternalInput")
    c128_d = nc.dram_tensor("c128", [128, NT], I32, kind="ExternalInput")
    out_d = nc.dram_tensor("out", [128, NT, 128], F32, kind="ExternalOutput")

    with tile.TileContext(nc) as tc:
        with (
            tc.tile_pool(name="pool", bufs=1) as pool,
            tc.tile_pool(name="ps", bufs=1, space="PSUM") as ps,
            tc.tile_pool(name="dpool", bufs=1, space="DRAM") as dpool,
        ):
            # ---- code decode (independent of x/basis; overlaps big DMAs)
            c16 = pool.tile([128, OPC // 16], I32)
            nc.sync.dma_start(out=c16[:], in_=c16_d[:])
            c128 = pool.tile([128, NT], I32)
            nc.sync.dma_start(out=c128[:], in_=c128_d[:])

            # gather indices: idx = codes & 255, int16, wrap-16 layout
            # (bitVec TSP ops cannot cast dtypes, so mask in i32 then
            # copy-cast to i16)
            idx32 = pool.tile([128, OPC // 16], I32)
            nc.vector.tensor_scalar(
                out=idx32[:], in0=c16[:],
                scalar1=255, scalar2=None, op0=AluOpType.bitwise_and,
            )
            idx16 = pool.tile([128, OPC // 16], I16)
            nc.vector.tensor_copy(out=idx16[:], in_=idx32[:])

            # radius: tanh(((codes >> 8) & 4095) / 4095)
            rq_i = pool.tile([128, NT], I32)
            nc.vector.tensor_scalar(
                out=rq_i[:], in0=c128[:],
                scalar1=8, scalar2=4095,
                op0=AluOpType.logical_shift_right, op1=AluOpType.bitwise_and,
            )
            rq = pool.tile([128, NT], F32)
            nc.vector.tensor_scalar_mul(
                out=rq[:], in0=rq_i[:], scalar1=1.0 / R_LEVELS
            )
            th = pool.tile([128, NT], F32)
            nc.scalar.activation(
                out=th[:], in_=rq[:],
                func=mybir.ActivationFunctionType.Tanh,
            )
            # sign: 1 - 2 * bit20
            sg_i = pool.tile([128, NT], I32)
            nc.vector.tensor_scalar(
                out=sg_i[:], in0=c128[:],
                scalar1=20, scalar2=1,
                op0=AluOpType.logical_shift_right, op1=AluOpType.bitwise_and,
            )
            sgn = pool.tile([128, NT], F32)
            nc.vector.tensor_scalar(
                out=sgn[:], in0=sg_i[:],
                scalar1=-2.0, scalar2=1.0,
                op0=AluOpType.mult, op1=AluOpType.add,
            )
            scl = pool.tile([128, NT], F32)
            nc.vector.tensor_tensor(
                out=scl[:], in0=th[:], in1=sgn[:], op=AluOpType.mult,
            )

            # ---- stream x^T / basis^T, accumulate Z^T in PSUM
            xt_sb = pool.tile([128, IN_F], F32)
            bt_sb = pool.tile([128, 2 * IN_F], F32)
            z0 = ps.tile([128, 128], F32)
            z1 = ps.tile([128, 128], F32)
            for g in range(0, NK, DMA_CHUNK):
                ge = g + DMA_CHUNK
                nc.sync.dma_start(
                    out=bt_sb[:, g * 256:ge * 256], in_=bt_d[:, g * 256:ge * 256]
                )
                nc.sync.dma_start(
                    out=xt_sb[:, g * 128:ge * 128], in_=xt_d[:, g * 128:ge * 128]
                )
                for n in range(g, ge):
                    rhs = xt_sb[:, n * 128:(n + 1) * 128]
                    nc.tensor.matmul(
                        z0[:],
                        lhsT=bt_sb[:, n * 256:n * 256 + 128],
                        rhs=rhs,
                        start=(n == 0), stop=(n == NK - 1),
                    )
                    nc.tensor.matmul(
                        z1[:],
                        lhsT=bt_sb[:, n * 256 + 128:n * 256 + 256],
                        rhs=rhs,
                        start=(n == 0), stop=(n == NK - 1),
                    )

            # ---- Z^T -> SBUF -> DRAM scratch
            ztd = dpool.tile([BASIS, 128], F32)
            zt0 = pool.tile([128, 128], F32)
            nc.vector.tensor_copy(out=zt0[:], in_=z0[:])
            zt1 = pool.tile([128, 128], F32)
            nc.vector.tensor_copy(out=zt1[:], in_=z1[:])
            nc.sync.dma_start(out=ztd[0:128, :], in_=zt0[:])
            nc.sync.dma_start(out=ztd[128:256, :], in_=zt1[:])

            # ---- gather rows Z^T[idx[j], :] -> [128, NT, 128]
            g_sb = pool.tile([128, NT, 128], F32)
            nc.gpsimd.dma_gather(
                out_ap=g_sb[:],
                in_ap=ztd[:],
                idxs_ap=idx16[:],
                num_idxs=OPC,
                num_idxs_reg=OPC,
                elem_size=128,
            )

            # ---- per-row scale, store
            o_sb = pool.tile([128, NT, 128], F32)
            for t in range(NT):
                nc.vector.tensor_scalar_mul(
                    out=o_sb[:, t, :], in0=g_sb[:, t, :], scalar1=scl[:, t:t + 1]
                )
            nc.sync.dma_start(out=out_d[:], in_=o_sb[:])

    nc.compile()
    return nc


_NC = None


def _get_nc():
    global _NC
    if _NC is None:
        _NC = build_nc()
    return _NC


def make_in_maps(x, codes, basis):
    x = np.ascontiguousarray(x, dtype=np.float32)
    basis = np.ascontiguousarray(basis, dtype=np.float32)
    codes = np.ascontiguousarray(codes, dtype=np.int32)

    # xt[p, n*128 + m] = x[m, n*128 + p]
    xt = np.ascontiguousarray(
        x.reshape(BATCH, NK, 128).transpose(2, 1, 0).reshape(128, IN_F)
    )
    # bt[p, n*256 + o] = basis[o, n*128 + p]
    bt = np.ascontiguousarray(
        basis.reshape(BASIS, NK, 128).transpose(2, 1, 0).reshape(128, 2 * IN_F)
    )

    in_maps = []
    for c in range(N_CORES):
        sh = codes[c * OPC:(c + 1) * OPC]
        # wrap-16 layout replicated to all 128 partitions (8 groups of 16)
        c16 = np.ascontiguousarray(
            np.tile(sh.reshape(OPC // 16, 16).T, (8, 1))
        )
        # wrap-128 layout: c128[p, t] = codes[t*128 + p]
        c128 = np.ascontiguousarray(sh.reshape(NT, 128).T)
        in_maps.append({"xt": xt, "bt": bt, "c16": c16, "c128": c128})
    return in_maps


def assemble_output(results):
    y = np.empty((BATCH, OUT_F), dtype=np.float32)
    for c in range(N_CORES):
        o = results[c]["out"]  # [128, NT, 128]; o[p, t, m] = y[m, c*OPC + t*128 + p]
        y[:, c * OPC:(c + 1) * OPC] = o.transpose(2, 1, 0).reshape(BATCH, OPC)
    return y


def kernel(x, codes, basis):
    nc = _get_nc()
    in_maps = make_in_maps(x, codes, basis)
    res = run_bass_kernel_spmd(nc, in_maps, list(range(N_CORES)))
    return assemble_output(res.results)


if __name__ == "__main__":
    # quick smoke test against a numpy reference
    rng = np.random.default_rng(0)
    x = rng.standard_normal((BATCH, IN_F), dtype=np.float32)
    basis = (rng.standard_normal((BASIS, IN_F), dtype=np.float32) * 0.02).astype(
        np.float32
    )
    codes = rng.integers(0, 1 << 22, size=(OUT_F,), dtype=np.int32)
    y = kernel(x, codes, basis)

    idx = codes & 255
    r = ((codes >> 8) & 4095).astype(np.float32) / R_LEVELS
    sign = np.where(((codes >> 20) & 1) == 1, -1.0, 1.0).astype(np.float32)
    scale = sign * np.tanh(r)
    W = scale[:, None] * basis[idx]
    y_ref = x @ W.T
    err = np.linalg.norm(y - y_ref) / np.linalg.norm(y_ref)
    print("rel err:", err)


# revision 4
# speedup vs baseline: 1.4150x; 1.4150x over previous
"""Trainium2 Bass kernel for nn_BitfieldLinear (vq_codebook).

Reference computation:
    idx   = codes & 0xFF            (basis row, 256 entries)
    r_q   = (codes >> 8) & 0xFFF
    sign  = bit20 ? -1 : +1
    scale = sign * tanh(r_q / 4095)
    W     = scale[:, None] * basis[idx]        # [8192, 4096]
    y     = x @ W.T                            # [128, 8192]

Key factorization (never materialize the 128MB W):
    Z = x @ basis.T                            # [128, 256]  tiny matmul
    y[b, j] = scale[j] * Z[b, idx[j]]          # column gather + scale

The gather+scale is itself a matmul with a scaled one-hot matrix:
    G[k, j] = scale[j] * (idx[j] == k)         # [256, 1024] per core
    y_core  = Z @ G                            # [128, 1024]
Each one-hot column has a single nonzero, so the fp32 matmul result is
exactly scale[j] * Z[b, idx[j]] — no precision loss vs reference.

Sharding: out_features column-parallel across 8 cores (1024 codes per
core); x and basis replicated.  Per core:
    1. stream x^T / basis^T K-tiled (host pre-laid-out for contiguous
       DMA, two HWDGE rings, tapered final chunks), accumulate
       Z^T [256, 128] in PSUM over 64 fp32 matmuls
    2. decode codes on-chip (DVE bitops + ACT tanh); build G^T tiles
       with one tensor_scalar each ((iota == idx) * scale), PE-transpose
       into G — all hidden under the input stream
    3. y = Z^T.T @ G via 4 fp32 matmuls, store [128, 1024]
Host reassembles y by concatenating per-core outputs (pure layout).
"""

import sys

for _p in ("/opt/trn_rl_repo", "/opt/pypackages"):
    if _p not in sys.path:
        sys.path.insert(0, _p)

import numpy as np

import concourse.bacc as bacc
import concourse.mybir as mybir
import concourse.tile as tile
from concourse.alu_op_type import AluOpType
from concourse.bass_utils import run_bass_kernel_spmd

N_CORES = 8
BATCH = 128
IN_F = 4096
OUT_F = 8192
BASIS = 256
OPC = OUT_F // N_CORES      # 1024 output columns per core
NK = IN_F // 128            # 32 K-tiles
NT = OPC // 128             # 8 code-tiles per core
R_LEVELS = 4095.0

F32 = mybir.dt.float32
I32 = mybir.dt.int32

# K-tiles per input DMA chunk, tapered so the PE tail after the last
# chunk stays small
DMA_CHUNKS = [8, 8, 8, 4, 2, 1, 1]
assert sum(DMA_CHUNKS) == NK


def build_nc():
    nc = bacc.Bacc(
        "TRN2",
        target_bir_lowering=False,
        debug=False,
        num_devices=N_CORES,
    )

    xt_d = nc.dram_tensor("xt", [128, IN_F], F32, kind="ExternalInput")
    bt_d = nc.dram_tensor("bt", [128, 2 * IN_F], F32, kind="ExternalInput")
    c128_d = nc.dram_tensor("c128", [128, NT], I32, kind="ExternalInput")
    iota_d = nc.dram_tensor("iota", [128, BASIS], F32, kind="ExternalInput")
    ident_d = nc.dram_tensor("ident", [128, 128], F32, kind="ExternalInput")
    out_d = nc.dram_tensor("out", [128, OPC], F32, kind="ExternalOutput")

    with tile.TileContext(nc) as tc:
        with (
            tc.tile_pool(name="pool", bufs=1) as pool,
            tc.tile_pool(name="zps", bufs=1, space="PSUM") as zps,
            tc.tile_pool(name="tps", bufs=2, space="PSUM") as tps,
            tc.tile_pool(name="yps", bufs=1, space="PSUM") as yps,
        ):
            # ---- small inputs (decode + constants); overlap big DMAs
            c128 = pool.tile([128, NT], I32)
            nc.sync.dma_start(out=c128[:], in_=c128_d[:])
            iota_bc = pool.tile([128, BASIS], F32)
            nc.sync.dma_start(out=iota_bc[:], in_=iota_d[:])
            ident = pool.tile([128, 128], F32)
            nc.sync.dma_start(out=ident[:], in_=ident_d[:])

            # ---- decode codes -> idx (f32), scale (f32), both [128, NT]
            # (bitVec TSP ops cannot cast dtypes: mask in i32, then cast
            # via fp-ALU mult)
            idx_i = pool.tile([128, NT], I32)
            nc.vector.tensor_scalar(
                out=idx_i[:], in0=c128[:],
                scalar1=255, scalar2=None, op0=AluOpType.bitwise_and,
            )
            idx_f = pool.tile([128, NT], F32)
            nc.vector.tensor_scalar_mul(out=idx_f[:], in0=idx_i[:], scalar1=1.0)

            rq_i = pool.tile([128, NT], I32)
            nc.vector.tensor_scalar(
                out=rq_i[:], in0=c128[:],
                scalar1=8, scalar2=4095,
                op0=AluOpType.logical_shift_right, op1=AluOpType.bitwise_and,
            )
            rq = pool.tile([128, NT], F32)
            nc.vector.tensor_scalar_mul(
                out=rq[:], in0=rq_i[:], scalar1=1.0 / R_LEVELS
            )
            th = pool.tile([128, NT], F32)
            nc.scalar.activation(
                out=th[:], in_=rq[:], func=mybir.ActivationFunctionType.Tanh,
            )
            sg_i = pool.tile([128, NT], I32)
            nc.vector.tensor_scalar(
                out=sg_i[:], in0=c128[:],
                scalar1=20, scalar2=1,
                op0=AluOpType.logical_shift_right, op1=AluOpType.bitwise_and,
            )
            sgn = pool.tile([128, NT], F32)
            nc.vector.tensor_scalar(
                out=sgn[:], in0=sg_i[:],
                scalar1=-2.0, scalar2=1.0,
                op0=AluOpType.mult, op1=AluOpType.add,
            )
            scl = pool.tile([128, NT], F32)
            nc.vector.tensor_tensor(
                out=scl[:], in0=th[:], in1=sgn[:], op=AluOpType.mult,
            )

            # ---- G^T tiles: gt[t][p, k] = scale[t*128+p] * (idx[t*128+p]==k)
            # one dual-op tensor_scalar per tile, then PE-transpose into G
            # G_sb[h][k', t*128+j'] with k = h*128+k'
            g_sb = [pool.tile([128, OPC], F32, tag=f"g{h}", name=f"g_sb{h}") for h in range(2)]
            for t in range(NT):
                gt = pool.tile([128, BASIS], F32, tag="gt")
                nc.vector.tensor_scalar(
                    out=gt[:], in0=iota_bc[:],
                    scalar1=idx_f[:, t:t + 1], scalar2=scl[:, t:t + 1],
                    op0=AluOpType.is_equal, op1=AluOpType.mult,
                )
                for h in range(2):
                    tp = tps.tile([128, 128], F32, tag="tp")
                    nc.tensor.transpose(
                        out=tp[:], in_=gt[:, h * 128:(h + 1) * 128],
                        identity=ident[:],
                    )
                    nc.vector.tensor_copy(
                        out=g_sb[h][:, t * 128:(t + 1) * 128], in_=tp[:]
                    )

            # ---- stream x^T / basis^T on both HWDGE rings, accumulate
            # Z^T [256, 128] in PSUM
            xt_sb = pool.tile([128, IN_F], F32)
            bt_sb = pool.tile([128, 2 * IN_F], F32)
            z0 = zps.tile([128, 128], F32, tag="z0")
            z1 = zps.tile([128, 128], F32, tag="z1")
            g = 0
            for ch in DMA_CHUNKS:
                ge = g + ch
                nc.sync.dma_start(
                    out=bt_sb[:, g * 256:ge * 256], in_=bt_d[:, g * 256:ge * 256]
                )
                nc.scalar.dma_start(
                    out=xt_sb[:, g * 128:ge * 128], in_=xt_d[:, g * 128:ge * 128]
                )
                for n in range(g, ge):
                    rhs = xt_sb[:, n * 128:(n + 1) * 128]
                    nc.tensor.matmul(
                        z0[:],
                        lhsT=bt_sb[:, n * 256:n * 256 + 128],
                        rhs=rhs,
                        start=(n == 0), stop=(n == NK - 1),
                    )
                    nc.tensor.matmul(
                        z1[:],
                        lhsT=bt_sb[:, n * 256 + 128:n * 256 + 256],
                        rhs=rhs,
                        start=(n == 0), stop=(n == NK - 1),
                    )
                g = ge

            zt = [pool.tile([128, 128], F32, tag=f"zt{h}", name=f"zt{h}") for h in range(2)]
            nc.vector.tensor_copy(out=zt[0][:], in_=z0[:])
            nc.vector.tensor_copy(out=zt[1][:], in_=z1[:])

            # ---- y = Z^T.T @ G, two N-chunks of 512, store each asap
            for nch in range(2):
                y_ps = yps.tile([128, 512], F32, tag=f"y{nch}")
                for h in range(2):
                    nc.tensor.matmul(
                        y_ps[:],
                        lhsT=zt[h][:],
                        rhs=g_sb[h][:, nch * 512:(nch + 1) * 512],
                        start=(h == 0), stop=(h == 1),
                    )
                y_sb = pool.tile([128, 512], F32, tag=f"ysb{nch}")
                nc.vector.tensor_copy(out=y_sb[:], in_=y_ps[:])
                nc.sync.dma_start(
                    out=out_d[:, nch * 512:(nch + 1) * 512], in_=y_sb[:]
                )

    nc.compile()
    return nc


_NC = None


def _get_nc():
    global _NC
    if _NC is None:
        _NC = build_nc()
    return _NC


def make_in_maps(x, codes, basis):
    x = np.ascontiguousarray(x, dtype=np.float32)
    basis = np.ascontiguousarray(basis, dtype=np.float32)
    codes = np.ascontiguousarray(codes, dtype=np.int32)

    # xt[p, n*128 + m] = x[m, n*128 + p]
    xt = np.ascontiguousarray(
        x.reshape(BATCH, NK, 128).transpose(2, 1, 0).reshape(128, IN_F)
    )
    # bt[p, n*256 + o] = basis[o, n*128 + p]
    bt = np.ascontiguousarray(
        basis.reshape(BASIS, NK, 128).transpose(2, 1, 0).reshape(128, 2 * IN_F)
    )
    iota = np.ascontiguousarray(
        np.tile(np.arange(BASIS, dtype=np.float32), (128, 1))
    )
    ident = np.eye(128, dtype=np.float32)

    in_maps = []
    for c in range(N_CORES):
        sh = codes[c * OPC:(c + 1) * OPC]
        # wrap-128 layout: c128[p, t] = codes[t*128 + p]
        c128 = np.ascontiguousarray(sh.reshape(NT, 128).T)
        in_maps.append(
            {"xt": xt, "bt": bt, "c128": c128, "iota": iota, "ident": ident}
        )
    return in_maps


def assemble_output(results):
    return np.concatenate(
        [results[c]["out"] for c in range(N_CORES)], axis=1
    ).astype(np.float32)


def kernel(x, codes, basis):
    nc = _get_nc()
    in_maps = make_in_maps(x, codes, basis)
    res = run_bass_kernel_spmd(nc, in_maps, list(range(N_CORES)))
    return assemble_output(res.results)


if __name__ == "__main__":
    rng = np.random.default_rng(0)
    x = rng.standard_normal((BATCH, IN_F), dtype=np.float32)
    basis = (rng.standard_normal((BASIS, IN_F)) * 0.02).astype(np.float32)
    codes = rng.integers(0, 1 << 22, size=(OUT_F,), dtype=np.int32)
    y = kernel(x, codes, basis)

    idx = codes & 255
    r = ((codes >> 8) & 4095).astype(np.float32) / R_LEVELS
    sign = np.where(((codes >> 20) & 1) == 1, -1.0, 1.0).astype(np.float32)
    scale = sign * np.tanh(r)
    W = scale[:, None] * basis[idx]
    y_ref = x @ W.T
    err = np.linalg.norm(y - y_ref) / np.linalg.norm(y_ref)
    print("rel err:", err)


# revision 8
# speedup vs baseline: 1.4509x; 1.0254x over previous
"""Trainium2 Bass kernel for nn_BitfieldLinear (vq_codebook).

Reference computation:
    idx   = codes & 0xFF            (basis row, 256 entries)
    r_q   = (codes >> 8) & 0xFFF
    sign  = bit20 ? -1 : +1
    scale = sign * tanh(r_q / 4095)
    W     = scale[:, None] * basis[idx]        # [8192, 4096]
    y     = x @ W.T                            # [128, 8192]

Key factorization (never materialize the 128MB W):
    Z = x @ basis.T                            # [128, 256]  tiny matmul
    y[b, j] = scale[j] * Z[b, idx[j]]          # column gather + scale

The gather+scale is itself a matmul with a scaled one-hot matrix:
    G[k, j] = scale[j] * (idx[j] == k)         # [256, 1024] per core
    y_core  = Z @ G                            # [128, 1024]
Each one-hot column has a single nonzero, so the fp32 matmul result is
exactly scale[j] * Z[b, idx[j]] — no precision loss vs reference.

Sharding: out_features column-parallel across 8 cores (1024 codes per
core); x and basis replicated.  Per core:
    1. stream x^T / basis^T K-tiled (host pre-laid-out for contiguous
       DMA, two HWDGE rings, tapered final chunks), accumulate
       Z^T [256, 128] in PSUM over 64 fp32 matmuls
    2. decode codes on-chip (DVE bitops + ACT tanh); build G^T tiles
       with one tensor_scalar each ((iota == idx) * scale), PE-transpose
       into G — all hidden under the input stream
    3. y = Z^T.T @ G via 4 fp32 matmuls, store [128, 1024]
Host reassembles y by concatenating per-core outputs (pure layout).
"""

import sys

for _p in ("/opt/trn_rl_repo", "/opt/pypackages"):
    if _p not in sys.path:
        sys.path.insert(0, _p)

import numpy as np

import concourse.bacc as bacc
import concourse.mybir as mybir
import concourse.tile as tile
from concourse.alu_op_type import AluOpType
from concourse.bass_utils import run_bass_kernel_spmd

N_CORES = 8
BATCH = 128
IN_F = 4096
OUT_F = 8192
BASIS = 256
OPC = OUT_F // N_CORES      # 1024 output columns per core
NK = IN_F // 128            # 32 K-tiles
NT = OPC // 128             # 8 code-tiles per core
R_LEVELS = 4095.0

F32 = mybir.dt.float32
F32R = mybir.dt.float32r
I32 = mybir.dt.int32

# K-tiles per input DMA chunk, tapered so the PE tail after the last
# chunk stays small
DMA_CHUNKS = [8, 8, 8, 4, 2, 1, 1]
assert sum(DMA_CHUNKS) == NK


def build_nc():
    nc = bacc.Bacc(
        "TRN2",
        target_bir_lowering=False,
        debug=False,
        num_devices=N_CORES,
    )

    xt_d = nc.dram_tensor("xt", [128, IN_F], F32, kind="ExternalInput")
    bt_d = nc.dram_tensor("bt", [128, 2 * IN_F], F32, kind="ExternalInput")
    c128_d = nc.dram_tensor("c128", [128, NT], I32, kind="ExternalInput")
    iota_d = nc.dram_tensor("iota", [128, BASIS], F32, kind="ExternalInput")
    ident_d = nc.dram_tensor("ident", [128, 128], F32, kind="ExternalInput")
    out_d = nc.dram_tensor("out", [128, OPC], F32, kind="ExternalOutput")

    with tile.TileContext(nc) as tc:
        with (
            tc.tile_pool(name="pool", bufs=1) as pool,
            tc.tile_pool(name="zps", bufs=1, space="PSUM") as zps,
            tc.tile_pool(name="tps", bufs=2, space="PSUM") as tps,
            tc.tile_pool(name="yps", bufs=1, space="PSUM") as yps,
        ):
            # ---- small inputs (decode + constants) on the SWDGE ring so
            # the two HWDGE rings start streaming x/basis immediately
            c128 = pool.tile([128, NT], I32)
            nc.gpsimd.dma_start(out=c128[:], in_=c128_d[:])
            iota_bc = pool.tile([128, BASIS], F32)
            nc.gpsimd.dma_start(out=iota_bc[:], in_=iota_d[:])
            ident = pool.tile([128, 128], F32)
            nc.gpsimd.dma_start(out=ident[:], in_=ident_d[:])

            # ---- decode codes -> idx (f32), scale (f32), both [128, NT]
            # (bitVec TSP ops cannot cast dtypes: mask in i32, then cast
            # via fp-ALU mult)
            idx_i = pool.tile([128, NT], I32)
            nc.vector.tensor_scalar(
                out=idx_i[:], in0=c128[:],
                scalar1=255, scalar2=None, op0=AluOpType.bitwise_and,
            )
            idx_f = pool.tile([128, NT], F32)
            nc.vector.tensor_scalar_mul(out=idx_f[:], in0=idx_i[:], scalar1=1.0)

            rq_i = pool.tile([128, NT], I32)
            nc.vector.tensor_scalar(
                out=rq_i[:], in0=c128[:],
                scalar1=8, scalar2=4095,
                op0=AluOpType.logical_shift_right, op1=AluOpType.bitwise_and,
            )
            rq = pool.tile([128, NT], F32)
            nc.vector.tensor_scalar_mul(
                out=rq[:], in0=rq_i[:], scalar1=1.0 / R_LEVELS
            )
            th = pool.tile([128, NT], F32)
            nc.scalar.activation(
                out=th[:], in_=rq[:], func=mybir.ActivationFunctionType.Tanh,
            )
            sg_i = pool.tile([128, NT], I32)
            nc.vector.tensor_scalar(
                out=sg_i[:], in0=c128[:],
                scalar1=20, scalar2=1,
                op0=AluOpType.logical_shift_right, op1=AluOpType.bitwise_and,
            )
            sgn = pool.tile([128, NT], F32)
            nc.vector.tensor_scalar(
                out=sgn[:], in0=sg_i[:],
                scalar1=-2.0, scalar2=1.0,
                op0=AluOpType.mult, op1=AluOpType.add,
            )
            scl = pool.tile([128, NT], F32)
            nc.vector.tensor_tensor(
                out=scl[:], in0=th[:], in1=sgn[:], op=AluOpType.mult,
            )

            # ---- G^T tiles: gt[t][p, k] = scale[t*128+p] * (idx[t*128+p]==k)
            # one dual-op tensor_scalar per tile, then PE-transpose into G
            # G_sb[h][k', t*128+j'] with k = h*128+k'
            g_sb = [pool.tile([128, OPC], F32R, tag=f"g{h}", name=f"g_sb{h}") for h in range(2)]
            for t in range(NT):
                gt = pool.tile([128, BASIS], F32, tag="gt")
                nc.vector.tensor_scalar(
                    out=gt[:], in0=iota_bc[:],
                    scalar1=idx_f[:, t:t + 1], scalar2=scl[:, t:t + 1],
                    op0=AluOpType.is_equal, op1=AluOpType.mult,
                )
                for h in range(2):
                    tp = tps.tile([128, 128], F32, tag="tp")
                    nc.tensor.transpose(
                        out=tp[:], in_=gt[:, h * 128:(h + 1) * 128],
                        identity=ident[:],
                    )
                    nc.vector.tensor_copy(
                        out=g_sb[h][:, t * 128:(t + 1) * 128], in_=tp[:]
                    )

            # ---- stream x^T / basis^T on both HWDGE rings, accumulate
            # Z [128b, 256o] in PSUM.  lhsT = x^T tile (N=256 moving keeps
            # the PE matmul-bound instead of LDWEIGHTS-bound at fp32)
            xt_sb = pool.tile([128, IN_F], F32)
            bt_sb = pool.tile([128, 2 * IN_F], F32)
            z_ps = zps.tile([128, BASIS], F32, tag="z")
            g = 0
            for ch in DMA_CHUNKS:
                ge = g + ch
                nc.sync.dma_start(
                    out=bt_sb[:, g * 256:ge * 256], in_=bt_d[:, g * 256:ge * 256]
                )
                nc.scalar.dma_start(
                    out=xt_sb[:, g * 128:ge * 128], in_=xt_d[:, g * 128:ge * 128]
                )
                for n in range(g, ge):
                    nc.tensor.matmul(
                        z_ps[:],
                        lhsT=xt_sb[:, n * 128:(n + 1) * 128],
                        rhs=bt_sb[:, n * 256:(n + 1) * 256],
                        start=(n == 0), stop=(n == NK - 1),
                    )
                g = ge

            # Z -> SBUF, PE-transpose into Z^T chunks for the y matmul
            z_sb = pool.tile([128, BASIS], F32)
            nc.vector.tensor_copy(out=z_sb[:], in_=z_ps[:])
            zt = [pool.tile([128, 128], F32R, tag=f"zt{h}", name=f"zt{h}") for h in range(2)]
            for h in range(2):
                ztp = tps.tile([128, 128], F32, tag="tp")
                nc.tensor.transpose(
                    out=ztp[:], in_=z_sb[:, h * 128:(h + 1) * 128],
                    identity=ident[:],
                )
                if h == 0:
                    nc.vector.tensor_copy(out=zt[h][:], in_=ztp[:])
                else:
                    nc.scalar.copy(out=zt[h][:], in_=ztp[:])

            # ---- y = Z^T.T @ G, two N-chunks of 512 (fp32r: each one-hot
            # column is a single product, so precision loss is negligible),
            # store each as soon as its PSUM copy lands
            for nch in range(2):
                y_ps = yps.tile([128, 512], F32, tag=f"y{nch}", name=f"y_ps{nch}")
                for h in range(2):
                    nc.tensor.matmul(
                        y_ps[:],
                        lhsT=zt[h][:],
                        rhs=g_sb[h][:, nch * 512:(nch + 1) * 512],
                        start=(h == 0), stop=(h == 1),
                    )
                y_sb = pool.tile([128, 512], F32, tag=f"ysb{nch}", name=f"y_sb{nch}")
                if nch == 0:
                    nc.vector.tensor_copy(out=y_sb[:], in_=y_ps[:])
                else:
                    nc.scalar.copy(out=y_sb[:], in_=y_ps[:])
                nc.sync.dma_start(
                    out=out_d[:, nch * 512:(nch + 1) * 512], in_=y_sb[:]
                )

    nc.compile()
    return nc


_NC = None


def _get_nc():
    global _NC
    if _NC is None:
        _NC = build_nc()
    return _NC


def make_in_maps(x, codes, basis):
    x = np.ascontiguousarray(x, dtype=np.float32)
    basis = np.ascontiguousarray(basis, dtype=np.float32)
    codes = np.ascontiguousarray(codes, dtype=np.int32)

    # xt[p, n*128 + m] = x[m, n*128 + p]
    xt = np.ascontiguousarray(
        x.reshape(BATCH, NK, 128).transpose(2, 1, 0).reshape(128, IN_F)
    )
    # bt[p, n*256 + o] = basis[o, n*128 + p]
    bt = np.ascontiguousarray(
        basis.reshape(BASIS, NK, 128).transpose(2, 1, 0).reshape(128, 2 * IN_F)
    )
    iota = np.ascontiguousarray(
        np.tile(np.arange(BASIS, dtype=np.float32), (128, 1))
    )
    ident = np.eye(128, dtype=np.float32)

    in_maps = []
    for c in range(N_CORES):
        sh = codes[c * OPC:(c + 1) * OPC]
        # wrap-128 layout: c128[p, t] = codes[t*128 + p]
        c128 = np.ascontiguousarray(sh.reshape(NT, 128).T)
        in_maps.append(
            {"xt": xt, "bt": bt, "c128": c128, "iota": iota, "ident": ident}
        )
    return in_maps


def assemble_output(results):
    return np.concatenate(
        [results[c]["out"] for c in range(N_CORES)], axis=1
    ).astype(np.float32)


def kernel(x, codes, basis):
    nc = _get_nc()
    in_maps = make_in_maps(x, codes, basis)
    res = run_bass_kernel_spmd(nc, in_maps, list(range(N_CORES)))
    return assemble_output(res.results)


if __name__ == "__main__":
    rng = np.random.default_rng(0)
    x = rng.standard_normal((BATCH, IN_F), dtype=np.float32)
    basis = (rng.standard_normal((BASIS, IN_F)) * 0.02).astype(np.float32)
    codes = rng.integers(0, 1 << 22, size=(OUT_F,), dtype=np.int32)
    y = kernel(x, codes, basis)

    idx = codes & 255
    r = ((codes >> 8) & 4095).astype(np.float32) / R_LEVELS
    sign = np.where(((codes >> 20) & 1) == 1, -1.0, 1.0).astype(np.float32)
    scale = sign * np.tanh(r)
    W = scale[:, None] * basis[idx]
    y_ref = x @ W.T
    err = np.linalg.norm(y - y_ref) / np.linalg.norm(y_ref)
    print("rel err:", err)
